# revision 19
# baseline (speedup 1.0000x reference)
"""Distributed Trainium2 kernel for CustomMultiHeadAttentionStoich.

Sharding (8 cores): core c = (batch b=c//4, group rank p=c%4).
Each core computes attention for its 512-query slice over ALL 16 heads.

K/V are HEAD-sharded for projection: core p projects heads 4p..4p+3 of
K^T [256, 2048] and V [2048, 256]. The gather of remote heads runs as TWO
pipelined AllGather waves (2 heads/rank each; K^T [128, 2048] and V
[2048, 128] packed into one pow2 1MiB buffer per wave -> Mesh algorithm),
so wave-A heads can start attention while wave B is still on the wire.
Additionally every core redundantly projects heads 0..3 locally (rank 0's
share, SPMD-uniform), so the first 4 attention slots run before any
collective completes. Attention slot order = [loc 0..3, waveA blocks 1..3,
waveB blocks 1..3]; the host permutes Wq/bq/qfeat/Wo per-head blocks to
match, so the device program is identical on every core.

The stoichiometric bias alpha_pos*relu(clamp(fk-fq)) + alpha_neg*min(clamp,0)
is a rank-2x16 SVD feature expansion of the piecewise-linear kernel of
(fk - fq), injected as extra contraction rows of the scores matmul.

Scores are computed in [key, query] layout; softmax denominators come from a
fused ones-column of the V tile through the AV matmul. exp() runs on the
scalar engine over [128, 1024] two-bank PSUM supertiles; the scalar engine
does only exp. The output projection runs incrementally per head-pair
(contraction 128) into an SBUF fp32 accumulator.
"""

import sys

sys.path.insert(0, "/opt/trn_rl_repo")

import numpy as np
import ml_dtypes

BF = ml_dtypes.bfloat16

B, T, D, H, DH = 2, 2048, 1024, 16, 64
NCORES = 8
TQ = 512  # queries per core
R = 16  # SVD rank per clamp-kernel half
AUG = 2 * R
CP = DH + AUG  # contraction rows for the scores matmul
NGRID = 1024  # SVD grid
KC = T // 128  # 16 key chunks
VG = DH + 1  # V tile columns per head incl. ones column
HG = 4  # heads per core for the sharded K/V projection
N_LOC = 4  # leading slots served by the redundant local projection

# slot -> head: [loc heads 0..3 | waveA blocks 1..3 | waveB blocks 1..3]
PERM = [0, 1, 2, 3, 4, 5, 8, 9, 12, 13, 6, 7, 10, 11, 14, 15]

_state = {}


def _features():
    """Rank-R SVD features of rc(x,y)=clip(x-y,0,0.2) on [0,1]^2."""
    if "grid" not in _state:
        g = (np.arange(NGRID) + 0.5) / NGRID
        M = np.clip(g[:, None] - g[None, :], 0.0, 0.2)
        U, S, Vt = np.linalg.svd(M, full_matrices=False)
        sc = np.sqrt(S[:R] * NGRID)
        _state["grid"] = g
        _state["phi"] = (U[:, :R] * sc).astype(np.float64)  # [NGRID, R] phi_j(x)
        _state["psi"] = (Vt[:R].T * sc).astype(np.float64)  # [NGRID, R] psi_j(y)
    return _state["grid"], _state["phi"], _state["psi"]


def _ev(tab, x):
    g = _state["grid"]
    return np.stack([np.interp(x, g, tab[:, j]) for j in range(R)])


def _build():
    if "nc" in _state:
        return _state["nc"]

    import concourse.bass as bass
    import concourse.mybir as mybir
    import concourse.tile as tile
    from concourse import bacc

    dt = mybir.dt
    ts = bass.ts
    ds = bass.ds

    nc = bacc.Bacc(
        "TRN2",
        target_bir_lowering=False,
        debug=False,
        num_devices=NCORES,
    )

    # ---- kernel I/O (per-core shards; host pre-slices) ----
    xqT = nc.dram_tensor("xqT", [D, TQ], dt.bfloat16, kind="ExternalInput").ap()
    xkT = nc.dram_tensor("xkT", [D, T], dt.bfloat16, kind="ExternalInput").ap()
    xvT = nc.dram_tensor("xvT", [D, T], dt.bfloat16, kind="ExternalInput").ap()
    wqT = nc.dram_tensor("wqT", [D, D], dt.bfloat16, kind="ExternalInput").ap()
    wkA = nc.dram_tensor("wkA", [D, HG * DH], dt.bfloat16, kind="ExternalInput").ap()
    wkL = nc.dram_tensor("wkL", [D, N_LOC * DH], dt.bfloat16, kind="ExternalInput").ap()
    wvA = nc.dram_tensor("wvA", [D, HG * DH], dt.bfloat16, kind="ExternalInput").ap()
    wvL = nc.dram_tensor("wvL", [D, N_LOC * DH], dt.bfloat16, kind="ExternalInput").ap()
    woT = nc.dram_tensor("woT", [D, D], dt.bfloat16, kind="ExternalInput").ap()
    bqE = nc.dram_tensor("bq", [D, 1], dt.float32, kind="ExternalInput").ap()
    bkAE = nc.dram_tensor("bkA", [HG * DH, 1], dt.float32, kind="ExternalInput").ap()
    bkLE = nc.dram_tensor("bkL", [N_LOC * DH, 1], dt.float32, kind="ExternalInput").ap()
    bvAE = nc.dram_tensor("bvA", [1, HG * DH], dt.bfloat16, kind="ExternalInput").ap()
    bvLE = nc.dram_tensor("bvL", [1, N_LOC * DH], dt.bfloat16, kind="ExternalInput").ap()
    boE = nc.dram_tensor("bo", [1, D], dt.bfloat16, kind="ExternalInput").ap()
    kfE = nc.dram_tensor("kfeat", [AUG, T], dt.bfloat16, kind="ExternalInput").ap()
    qfE = nc.dram_tensor("qfeat", [H * AUG, TQ], dt.bfloat16, kind="ExternalInput").ap()
    outE = nc.dram_tensor("out", [TQ, D], dt.float32, kind="ExternalOutput").ap()

    Exp = mybir.ActivationFunctionType.Exp
    RG = [[0, 1, 2, 3], [4, 5, 6, 7]]
    Bypass = mybir.AluOpType.bypass
    Mult = mybir.AluOpType.mult
    Add = mybir.AluOpType.add

    with tile.TileContext(nc) as tc:
        with (
            tc.tile_pool(name="dram", bufs=1, space="DRAM") as dram,
            tc.tile_pool(name="consts", bufs=1) as consts,
            tc.tile_pool(name="ehat", bufs=3) as ep,
            tc.tile_pool(name="stage", bufs=4) as stp,
            tc.tile_pool(name="rr", bufs=1) as rrp,
            tc.tile_pool(name="rb", bufs=2) as rbp,
            tc.tile_pool(name="aot", bufs=2) as aop,
            tc.tile_pool(name="psA", bufs=2, space="PSUM") as psA,
            tc.tile_pool(name="psS", bufs=2, space="PSUM") as psS,
            tc.tile_pool(name="psAV", bufs=2, space="PSUM") as psAV,
        ):
            # ---- DRAM scratch: per-wave packed K+V AllGather buffers ----
            # wave in: rows 0:128 = K^T (2 heads), rows 128:256 = V [2048, 128]
            # viewed flat; out = 4 rank blocks of the same layout.
            kv_in = [
                dram.tile([2 * 128, T], dt.bfloat16, tag=f"kvi{w}", name=f"kvi{w}")
                for w in range(2)
            ]
            kv_out = [
                dram.tile([8 * 128, T], dt.bfloat16, tag=f"kvo{w}", name=f"kvo{w}")
                for w in range(2)
            ]
            k_loc = dram.tile([N_LOC * DH, T], dt.bfloat16, tag="k_loc", name="k_loc")

            # ---- resident constants / working SBUF ----
            xk_sb = consts.tile([128, 8, T], dt.bfloat16, tag="xk", name="xk")
            xv_sb = consts.tile([128, 8, T], dt.bfloat16, tag="xv", name="xv")
            xq_sb = consts.tile([128, 8, TQ], dt.bfloat16, tag="xq", name="xq")
            wq_sb = consts.tile([128, 8, D], dt.bfloat16, tag="wq", name="wq")
            wkA_sb = consts.tile([128, 8, HG * DH], dt.bfloat16, tag="wka", name="wka")
            wkL_sb = consts.tile([128, 8, N_LOC * DH], dt.bfloat16, tag="wkl", name="wkl")
            wvA_sb = consts.tile([128, 8, HG * DH], dt.bfloat16, tag="wva", name="wva")
            wvL_sb = consts.tile([128, 8, N_LOC * DH], dt.bfloat16, tag="wvl", name="wvl")
            wo_sb = consts.tile([128, 8, D], dt.bfloat16, tag="wo", name="wo")
            bq_sb = consts.tile([128, 8, 1], dt.float32, tag="bq", name="bq")
            bkA_sb = consts.tile([128, 2, 1], dt.float32, tag="bka", name="bka")
            bkL_sb = consts.tile([128, 2, 1], dt.float32, tag="bkl", name="bkl")
            bvA_sb = consts.tile([1, HG * DH], dt.bfloat16, tag="bva", name="bva")
            bvL_sb = consts.tile([1, N_LOC * DH], dt.bfloat16, tag="bvl", name="bvl")
            bo_sb = consts.tile([1, D], dt.bfloat16, tag="bo", name="bo")
            ones_sb = consts.tile([1, 128], dt.bfloat16, tag="ones", name="ones")
            y_acc = consts.tile([128, 4, D], dt.float32, tag="yacc", name="yacc")

            kat = [
                consts.tile([CP, T], dt.bfloat16, tag=f"kat{i}", name=f"kat{i}")
                for i in range(3)
            ]
            vtl = [
                consts.tile([128, KC, VG], dt.bfloat16, tag=f"vtl{i}", name=f"vtl{i}")
                for i in range(3)
            ]
            vtl_loc = [
                consts.tile([128, KC, VG], dt.bfloat16, tag=f"vtlL{i}", name=f"vtlL{i}")
                for i in range(N_LOC)
            ]
            qat = consts.tile([CP, H, TQ], dt.bfloat16, tag="qat", name="qat")

            # ---- dep-free input DMAs on the scalar-engine DGE pipe ----
            for a in range(4):
                nc.scalar.dma_start(
                    out=xk_sb[:, ds(2 * a, 2), :],
                    in_=xkT.rearrange("(a p) m -> p a m", p=128)[:, ds(2 * a, 2), :],
                )
            nc.scalar.dma_start(
                out=wkA_sb, in_=wkA.rearrange("(a p) m -> p a m", p=128)
            )
            nc.scalar.dma_start(
                out=bkA_sb, in_=bkAE.rearrange("(a p) o -> p a o", p=128)
            )
            nc.scalar.dma_start(out=bvA_sb, in_=bvAE)
            nc.vector.memset(ones_sb, 1.0)
            for a in range(4):
                nc.scalar.dma_start(
                    out=xv_sb[:, ds(2 * a, 2), :],
                    in_=xvT.rearrange("(a p) m -> p a m", p=128)[:, ds(2 * a, 2), :],
                )
            nc.scalar.dma_start(
                out=wvA_sb, in_=wvA.rearrange("(a p) m -> p a m", p=128)
            )

            # ---- K^T projection: rows = head dims, cols = keys ----
            # dc-th 128-row chunk goes to dst[dc] at row dst_row[dc].
            def kproj(w_sb, b_sb, dsts, dst_rows, dcs):
                for dc in dcs:
                    for tc_i in range(4):
                        ps = psA.tile([128, 512], dt.float32, tag="mm", name="mmk")
                        for kc in range(8):
                            nc.tensor.matmul(
                                ps,
                                lhsT=w_sb[:, kc, ts(dc, 128)],
                                rhs=xk_sb[:, kc, ts(tc_i, 512)],
                                start=(kc == 0),
                                stop=(kc == 7),
                            )
                        stg = stp.tile(
                            [128, 512], dt.bfloat16, tag="kstage", name="kstage"
                        )
                        nc.vector.tensor_scalar_add(stg, ps, b_sb[:, dc, :])
                        nc.sync.dma_start(
                            out=dsts[dc][ds(dst_rows[dc], 128), ts(tc_i, 512)],
                            in_=stg,
                        )

            # ---- V projection: rows = keys, cols = head dims [cl, cl+cw) ----
            def vproj(w_sb, bv_row, cl, cw, view, vrow):
                for tc_i in range(16):
                    ps = psA.tile([128, cw], dt.float32, tag="mm", name="mmv")
                    for kc in range(8):
                        nc.tensor.matmul(
                            ps,
                            lhsT=xv_sb[:, kc, ts(tc_i, 128)],
                            rhs=w_sb[:, kc, ds(cl, cw)],
                            start=(kc == 0),
                            stop=False,
                        )
                    nc.tensor.matmul(
                        ps,
                        lhsT=ones_sb[:, :],
                        rhs=bv_row[:, ds(cl, cw)],
                        start=False,
                        stop=True,
                    )
                    stg = stp.tile([128, cw], dt.bfloat16, tag="vstage", name="vstage")
                    nc.vector.tensor_copy(stg, ps)
                    nc.sync.dma_start(
                        out=view[ds(vrow + 128 * tc_i, 128), :], in_=stg
                    )

            # ---- P1: sharded K/V projections -> two AllGather waves ----
            vin_view = [
                kv_in[w].rearrange("r (s n) -> (r s) n", n=128) for w in range(2)
            ]
            kproj(wkA_sb, bkA_sb, (kv_in[0], kv_in[1]), (0, 0), (0,))
            vproj(wvA_sb, bvA_sb, 0, 128, vin_view[0], T)
            kproj(wkA_sb, bkA_sb, (kv_in[0], kv_in[1]), (0, 0), (1,))
            vproj(wvA_sb, bvA_sb, 128, 128, vin_view[1], T)

            # ---- remaining dep-free input DMAs on the scalar DGE pipe ----
            for (dst, srct) in (
                (wkL_sb, wkL),
                (wvL_sb, wvL),
                (wq_sb, wqT),
                (xq_sb, xqT),
                (wo_sb, woT),
            ):
                nc.scalar.dma_start(
                    out=dst, in_=srct.rearrange("(a p) m -> p a m", p=128)
                )
            nc.scalar.dma_start(
                out=bq_sb, in_=bqE.rearrange("(a p) o -> p a o", p=128)
            )
            nc.scalar.dma_start(
                out=bkL_sb, in_=bkLE.rearrange("(a p) o -> p a o", p=128)
            )
            nc.scalar.dma_start(out=bvL_sb, in_=bvLE)
            nc.scalar.dma_start(out=bo_sb, in_=boE)
            nc.scalar.dma_start(
                out=qat[DH:CP, :, :],
                in_=qfE.rearrange("(h r) q -> r h q", r=AUG),
            )
            for i in range(3):
                nc.scalar.dma_start(out=kat[i][DH:CP, :], in_=kfE)
                nc.vector.memset(vtl[i][:, :, DH:VG], 1.0)
            for i in range(N_LOC):
                nc.vector.memset(vtl_loc[i][:, :, DH:VG], 1.0)

            # ---- P2: redundant local projections of heads 0..3 ----
            kproj(wkL_sb, bkL_sb, (k_loc, k_loc), (0, 128), (0, 1))
            for tc_i in range(16):
                ps = psA.tile([128, N_LOC * DH], dt.float32, tag="mm", name="mmvl")
                for kc in range(8):
                    nc.tensor.matmul(
                        ps,
                        lhsT=xv_sb[:, kc, ts(tc_i, 128)],
                        rhs=wvL_sb[:, kc, :],
                        start=(kc == 0),
                        stop=False,
                    )
                nc.tensor.matmul(
                    ps, lhsT=ones_sb[:, :], rhs=bvL_sb, start=False, stop=True
                )
                for hh in range(N_LOC):
                    nc.vector.tensor_copy(
                        vtl_loc[hh][:, tc_i, 0:DH], ps[:, ds(DH * hh, DH)]
                    )

            # ---- P3: Q projection (wq columns already in slot order) ----
            for dc in range(8):
                ps = psA.tile([128, TQ], dt.float32, tag="mm", name="mmq")
                for kc in range(8):
                    nc.tensor.matmul(
                        ps,
                        lhsT=wq_sb[:, kc, ts(dc, 128)],
                        rhs=xq_sb[:, kc, :],
                        start=(kc == 0),
                        stop=(kc == 7),
                    )
                stg = stp.tile([128, TQ], dt.bfloat16, tag="qstage", name="qstage")
                nc.vector.tensor_scalar_add(stg, ps, bq_sb[:, dc, :])
                nc.sync.dma_start(out=qat[0:DH, 2 * dc, :], in_=stg[0:DH, :])
                nc.sync.dma_start(out=qat[0:DH, 2 * dc + 1, :], in_=stg[DH:128, :])

            nc.gpsimd.collective_compute(
                "AllGather",
                Bypass,
                ins=[kv_in[0].opt()],
                outs=[kv_out[0].opt()],
                replica_groups=RG,
            )
            nc.gpsimd.collective_compute(
                "AllGather",
                Bypass,
                ins=[kv_in[1].opt()],
                outs=[kv_out[1].opt()],
                replica_groups=RG,
            )
            vout_view = [
                kv_out[w].rearrange("r (s n) -> (r s) n", n=128) for w in range(2)
            ]

            # ---- P4: attention per slot ----
            # The out-projection of pair p is emitted in the middle of slot
            # 2p+2's supertile stream so the in-order PE queue never stalls
            # on the DVE/gpsimd normalization chain feeding cur_aot.
            def outproj(pair, aot_t, half=None):
                qcs = range(4) if half is None else range(2 * half, 2 * half + 2)
                for qc in qcs:
                    for mc in range(2):
                        ps_y = psA.tile([128, 512], dt.float32, tag="mm", name="mmy")
                        nc.tensor.matmul(
                            ps_y,
                            lhsT=aot_t[:, ts(qc, 128)],
                            rhs=wo_sb[:, pair, ts(mc, 512)],
                            start=True,
                            stop=(pair != 0),
                        )
                        if pair == 0:
                            nc.tensor.matmul(
                                ps_y,
                                lhsT=ones_sb[:, :],
                                rhs=bo_sb[:, ds(512 * mc, 512)],
                                start=False,
                                stop=True,
                            )
                            nc.vector.tensor_copy(
                                y_acc[:, qc, ds(512 * mc, 512)], ps_y
                            )
                        else:
                            nc.vector.tensor_tensor(
                                out=y_acc[:, qc, ds(512 * mc, 512)],
                                in0=ps_y,
                                in1=y_acc[:, qc, ds(512 * mc, 512)],
                                op=Add,
                            )

            cur_aot = None
            pending = None
            for j in range(H):
                ka = kat[j % 3]
                if j < N_LOC:
                    vt = vtl_loc[j]
                    nc.sync.dma_start(out=ka[0:DH, :], in_=k_loc[ds(DH * j, DH), :])
                else:
                    vt = vtl[j % 3]
                    w = 0 if j < 10 else 1
                    r = 1 + (j - (4 if w == 0 else 10)) // 2
                    m = (j - (4 if w == 0 else 10)) % 2
                    nc.sync.dma_start(
                        out=ka[0:DH, :],
                        in_=kv_out[w][ds(256 * r + DH * m, DH), :],
                    )
                    vsrc = vout_view[w][ds(4096 * r + T, T), ds(DH * m, DH)]
                    nc.sync.dma_start(
                        out=vt[:, :, 0:DH],
                        in_=vsrc.rearrange("(a p) m -> p a m", p=128),
                    )
                ps_av = psAV.tile([VG, TQ], dt.float32, tag="av", name="av")
                for i in range(KC // 2):
                    ss = psS.tile([128, 2 * TQ], dt.float32, tag="s", name="s")
                    nc.tensor.matmul(
                        ss[:, 0:TQ],
                        lhsT=ka[:, ts(2 * i, 128)],
                        rhs=qat[:, j, :],
                        start=True,
                        stop=True,
                    )
                    nc.tensor.matmul(
                        ss[:, TQ : 2 * TQ],
                        lhsT=ka[:, ts(2 * i + 1, 128)],
                        rhs=qat[:, j, :],
                        start=True,
                        stop=True,
                    )
                    eh = ep.tile([128, 2 * TQ], dt.bfloat16, tag="ehat", name="ehat")
                    nc.scalar.activation(eh, ss, Exp)
                    nc.tensor.matmul(
                        ps_av,
                        lhsT=vt[:, 2 * i, :],
                        rhs=eh[:, 0:TQ],
                        start=(i == 0),
                        stop=False,
                    )
                    nc.tensor.matmul(
                        ps_av,
                        lhsT=vt[:, 2 * i + 1, :],
                        rhs=eh[:, TQ : 2 * TQ],
                        start=False,
                        stop=(i == KC // 2 - 1),
                    )
                    if i in (2, 5) and pending is not None:
                        outproj(*pending, half=(i == 5))
                        if i == 5:
                            pending = None
                # normalization: DVE reciprocal + gpsimd broadcast + DVE mul
                den = rrp.tile([1, TQ], dt.float32, tag="den", name="den")
                nc.vector.tensor_copy(den, ps_av[ds(DH, 1), :])
                rcp = rrp.tile([1, TQ], dt.float32, tag="rcp", name="rcp")
                nc.vector.reciprocal_approx_fast(rcp, den)
                rcpb = rrp.tile([1, TQ], dt.bfloat16, tag="rcpb", name="rcpb")
                nc.vector.tensor_copy(rcpb, rcp)
                rb = rbp.tile([DH, TQ], dt.bfloat16, tag="rb", name="rb")
                nc.gpsimd.partition_broadcast(rb, rcpb)
                if j % 2 == 0:
                    cur_aot = aop.tile([128, TQ], dt.bfloat16, tag="ao", name="ao")
                nc.vector.tensor_tensor(
                    out=cur_aot[ds(DH * (j % 2), DH), :],
                    in0=ps_av[0:DH, :],
                    in1=rb,
                    op=Mult,
                )
                if j % 2 == 1:
                    pending = (j // 2, cur_aot)
            outproj(*pending)


            # ---- P5: output DMA ----
            nc.sync.dma_start(
                out=outE.rearrange("(a p) m -> p a m", p=128), in_=y_acc
            )

    nc.compile()
    _state["nc"] = nc
    return nc


def _make_in_maps(inputs):
    _features()
    gs = float(np.float32(inputs["gamma"])) * DH ** -0.5
    delta = float(np.float32(inputs["delta"]))
    ap_ = np.asarray(inputs["alpha_pos"], np.float64)
    an_ = np.asarray(inputs["alpha_neg"], np.float64)

    # permute per-head blocks of Wq/bq/qfeat/Wo into slot order
    wqT_n = np.asarray(inputs["Wq"], np.float64).T * gs
    bq_n = np.asarray(inputs["bq"], np.float64) * gs
    wqTh = np.concatenate([wqT_n[:, DH * h : DH * (h + 1)] for h in PERM], 1).astype(BF)
    bqh = np.concatenate(
        [bq_n[DH * h : DH * (h + 1)] for h in PERM], 0
    ).astype(np.float32)[:, None]
    woT_n = np.ascontiguousarray(np.asarray(inputs["Wo"]).T)
    woTh = np.concatenate([woT_n[DH * h : DH * (h + 1), :] for h in PERM], 0).astype(BF)

    wkT_full = np.ascontiguousarray(np.asarray(inputs["Wk"]).T)
    bk_full = np.asarray(inputs["bk"], np.float32)
    wvT_full = np.ascontiguousarray(np.asarray(inputs["Wv"]).T)
    bv_full = np.asarray(inputs["bv"], np.float32)
    boh = np.asarray(inputs["bo"], np.float32)[None, :].astype(BF)

    wkLh = np.ascontiguousarray(wkT_full[:, : N_LOC * DH]).astype(BF)
    bkLh = np.ascontiguousarray(bk_full[: N_LOC * DH])[:, None]
    wvLh = np.ascontiguousarray(wvT_full[:, : N_LOC * DH]).astype(BF)
    bvLh = bv_full[None, : N_LOC * DH].astype(BF)

    phi, psi = _state["phi"], _state["psi"]
    frac = np.asarray(inputs["frac"], np.float64)

    in_maps = []
    for c in range(NCORES):
        b, p = c // 4, c % 4
        fb = frac[b]
        fq = fb[TQ * p : TQ * (p + 1)]
        kfeat = np.concatenate([_ev(phi, fb), _ev(psi, fb)], 0).astype(BF)
        qfeat = np.zeros((H * AUG, TQ), np.float64)
        for s, h in enumerate(PERM):
            a_h = delta * ap_[h] / NGRID
            b_h = -delta * an_[h] / NGRID
            qfeat[AUG * s : AUG * s + R] = a_h * _ev(psi, fq)
            qfeat[AUG * s + R : AUG * (s + 1)] = b_h * _ev(phi, fq)
        qfeat = qfeat.astype(BF)

        xq = np.asarray(inputs["query"])[b, TQ * p : TQ * (p + 1)]
        sl = slice(HG * DH * p, HG * DH * (p + 1))
        in_maps.append(
            {
                "xqT": np.ascontiguousarray(xq.T).astype(BF),
                "xkT": np.ascontiguousarray(np.asarray(inputs["key"])[b].T).astype(BF),
                "xvT": np.ascontiguousarray(
                    np.asarray(inputs["value"])[b].T
                ).astype(BF),
                "wqT": wqTh,
                "wkA": np.ascontiguousarray(wkT_full[:, sl]).astype(BF),
                "wkL": wkLh,
                "wvA": np.ascontiguousarray(wvT_full[:, sl]).astype(BF),
                "wvL": wvLh,
                "woT": woTh,
                "bq": bqh,
                "bkA": np.ascontiguousarray(bk_full[sl])[:, None],
                "bkL": bkLh,
                "bvA": bv_full[None, sl].astype(BF),
                "bvL": bvLh,
                "bo": boh,
                "kfeat": kfeat,
                "qfeat": qfeat,
            }
        )
    return in_maps


def _run(inputs, trace=False, **kw):
    from concourse.bass_utils import run_bass_kernel_spmd

    nc = _build()
    in_maps = _make_in_maps(inputs)
    res = run_bass_kernel_spmd(
        nc, in_maps, core_ids=list(range(NCORES)), trace=trace, **kw
    )
    out = np.zeros((B, T, D), np.float32)
    for c in range(NCORES):
        b, p = c // 4, c % 4
        out[b, TQ * p : TQ * (p + 1)] = res.results[c]["out"]
    return out, res


def kernel(**inputs):
    out, _ = _run(inputs)
    return out


# revision 20
# speedup vs baseline: 1.0749x; 1.0749x over previous
"""Distributed Trainium2 kernel for CustomMultiHeadAttentionStoich.

Sharding (8 cores): core c = (batch b=c//4, group rank p=c%4).
Each core computes attention for its 512-query slice over ALL 16 heads.

K/V are HEAD-sharded for projection: core p projects heads 4p..4p+3 of
K^T [256, 2048] and V [2048, 256]. The gather of remote heads runs as TWO
pipelined AllGather waves (2 heads/rank each; K^T [128, 2048] and V
[2048, 128] packed into one pow2 1MiB buffer per wave -> Mesh algorithm),
so wave-A heads can start attention while wave B is still on the wire.
Additionally every core redundantly projects heads 0..3 locally (rank 0's
share, SPMD-uniform), so the first 4 attention slots run before any
collective completes. Attention slot order = [loc 0..3, waveA blocks 1..3,
waveB blocks 1..3]; the host permutes Wq/bq/qfeat/Wo per-head blocks to
match, so the device program is identical on every core.

The stoichiometric bias alpha_pos*relu(clamp(fk-fq)) + alpha_neg*min(clamp,0)
is a rank-2x16 SVD feature expansion of the piecewise-linear kernel of
(fk - fq), injected as extra contraction rows of the scores matmul.

Scores are computed in [key, query] layout; softmax denominators come from a
fused ones-column of the V tile through the AV matmul. exp() runs on the
scalar engine over [128, 1024] two-bank PSUM supertiles; normalization is
DVE reciprocal + gpsimd partition_broadcast + DVE multiply, so the scalar
engine does only exp. The output projection runs incrementally per head-pair
(contraction 128) into an SBUF fp32 accumulator, emitted one slot late so
the in-order PE queue never stalls on the normalization chain.
"""

import sys

sys.path.insert(0, "/opt/trn_rl_repo")

import numpy as np
import ml_dtypes

BF = ml_dtypes.bfloat16

B, T, D, H, DH = 2, 2048, 1024, 16, 64
NCORES = 8
TQ = 512  # queries per core
R = 16  # SVD rank per clamp-kernel half
AUG = 2 * R
CP = DH + AUG  # contraction rows for the scores matmul
NGRID = 1024  # SVD grid
KC = T // 128  # 16 key chunks
VG = DH + 1  # V tile columns per head incl. ones column
HG = 4  # heads per core for the sharded K/V projection
N_LOC = 4  # leading slots served by the redundant local projection

# slot -> head: [loc heads 0..3 | waveA blocks 1..3 | waveB blocks 1..3]
PERM = [0, 1, 2, 3, 4, 5, 8, 9, 12, 13, 6, 7, 10, 11, 14, 15]

_state = {}


def _features():
    """Rank-R SVD features of rc(x,y)=clip(x-y,0,0.2) on [0,1]^2."""
    if "grid" not in _state:
        g = (np.arange(NGRID) + 0.5) / NGRID
        M = np.clip(g[:, None] - g[None, :], 0.0, 0.2)
        U, S, Vt = np.linalg.svd(M, full_matrices=False)
        sc = np.sqrt(S[:R] * NGRID)
        _state["grid"] = g
        _state["phi"] = (U[:, :R] * sc).astype(np.float64)  # [NGRID, R] phi_j(x)
        _state["psi"] = (Vt[:R].T * sc).astype(np.float64)  # [NGRID, R] psi_j(y)
    return _state["grid"], _state["phi"], _state["psi"]


def _ev(tab, x):
    g = _state["grid"]
    return np.stack([np.interp(x, g, tab[:, j]) for j in range(R)])


def _build():
    if "nc" in _state:
        return _state["nc"]

    import concourse.bass as bass
    import concourse.mybir as mybir
    import concourse.tile as tile
    from concourse import bacc

    dt = mybir.dt
    ts = bass.ts
    ds = bass.ds

    nc = bacc.Bacc(
        "TRN2",
        target_bir_lowering=False,
        debug=False,
        num_devices=NCORES,
    )

    # ---- kernel I/O (per-core shards; host pre-slices) ----
    xqT = nc.dram_tensor("xqT", [D, TQ], dt.bfloat16, kind="ExternalInput").ap()
    xkT = nc.dram_tensor("xkT", [D, T], dt.bfloat16, kind="ExternalInput").ap()
    xvT = nc.dram_tensor("xvT", [D, T], dt.bfloat16, kind="ExternalInput").ap()
    wqT = nc.dram_tensor("wqT", [D, D], dt.bfloat16, kind="ExternalInput").ap()
    wkA = nc.dram_tensor("wkA", [D, HG * DH], dt.bfloat16, kind="ExternalInput").ap()
    wkL = nc.dram_tensor("wkL", [D, N_LOC * DH], dt.bfloat16, kind="ExternalInput").ap()
    wvA = nc.dram_tensor("wvA", [D, HG * DH], dt.bfloat16, kind="ExternalInput").ap()
    wvL = nc.dram_tensor("wvL", [D, N_LOC * DH], dt.bfloat16, kind="ExternalInput").ap()
    woT = nc.dram_tensor("woT", [D, D], dt.bfloat16, kind="ExternalInput").ap()
    bqE = nc.dram_tensor("bq", [D, 1], dt.float32, kind="ExternalInput").ap()
    bkAE = nc.dram_tensor("bkA", [HG * DH, 1], dt.float32, kind="ExternalInput").ap()
    bkLE = nc.dram_tensor("bkL", [N_LOC * DH, 1], dt.float32, kind="ExternalInput").ap()
    bvAE = nc.dram_tensor("bvA", [1, HG * DH], dt.bfloat16, kind="ExternalInput").ap()
    bvLE = nc.dram_tensor("bvL", [1, N_LOC * DH], dt.bfloat16, kind="ExternalInput").ap()
    boE = nc.dram_tensor("bo", [1, D], dt.bfloat16, kind="ExternalInput").ap()
    kfE = nc.dram_tensor("kfeat", [AUG, T], dt.bfloat16, kind="ExternalInput").ap()
    qfE = nc.dram_tensor("qfeat", [H * AUG, TQ], dt.bfloat16, kind="ExternalInput").ap()
    outE = nc.dram_tensor("out", [TQ, D], dt.float32, kind="ExternalOutput").ap()

    Exp = mybir.ActivationFunctionType.Exp
    RG = [[0, 1, 2, 3], [4, 5, 6, 7]]
    Bypass = mybir.AluOpType.bypass
    Mult = mybir.AluOpType.mult
    Add = mybir.AluOpType.add

    with tile.TileContext(nc) as tc:
        with (
            tc.tile_pool(name="dram", bufs=1, space="DRAM") as dram,
            tc.tile_pool(name="consts", bufs=1) as consts,
            tc.tile_pool(name="ehat", bufs=3) as ep,
            tc.tile_pool(name="stage", bufs=4) as stp,
            tc.tile_pool(name="rr", bufs=2) as rrp,
            tc.tile_pool(name="rb", bufs=2) as rbp,
            tc.tile_pool(name="aot", bufs=2) as aop,
            tc.tile_pool(name="psA", bufs=2, space="PSUM") as psA,
            tc.tile_pool(name="psS", bufs=2, space="PSUM") as psS,
            tc.tile_pool(name="psAV", bufs=2, space="PSUM") as psAV,
        ):
            # ---- DRAM scratch: per-wave packed K+V AllGather buffers ----
            # wave in: rows 0:128 = K^T (2 heads), rows 128:256 = V [2048, 128]
            # viewed flat; out = 4 rank blocks of the same layout.
            kv_in = [
                dram.tile([2 * 128, T], dt.bfloat16, tag=f"kvi{w}", name=f"kvi{w}")
                for w in range(2)
            ]
            kv_out = [
                dram.tile([8 * 128, T], dt.bfloat16, tag=f"kvo{w}", name=f"kvo{w}")
                for w in range(2)
            ]
            k_loc = dram.tile([N_LOC * DH, T], dt.bfloat16, tag="k_loc", name="k_loc")
            v_loc = dram.tile([T, N_LOC * DH], dt.bfloat16, tag="v_loc", name="v_loc")

            # ---- resident constants / working SBUF ----
            xk_sb = consts.tile([128, 8, T], dt.bfloat16, tag="xk", name="xk")
            xv_sb = consts.tile([128, 8, T], dt.bfloat16, tag="xv", name="xv")
            xq_sb = consts.tile([128, 8, TQ], dt.bfloat16, tag="xq", name="xq")
            wq_sb = consts.tile([128, 8, D], dt.bfloat16, tag="wq", name="wq")
            wkA_sb = consts.tile([128, 8, HG * DH], dt.bfloat16, tag="wka", name="wka")
            wkL_sb = consts.tile([128, 8, N_LOC * DH], dt.bfloat16, tag="wkl", name="wkl")
            wvA_sb = consts.tile([128, 8, HG * DH], dt.bfloat16, tag="wva", name="wva")
            wvL_sb = consts.tile([128, 8, N_LOC * DH], dt.bfloat16, tag="wvl", name="wvl")
            wo_sb = consts.tile([128, 8, D], dt.bfloat16, tag="wo", name="wo")
            bq_sb = consts.tile([128, 8, 1], dt.float32, tag="bq", name="bq")
            bkA_sb = consts.tile([128, 2, 1], dt.float32, tag="bka", name="bka")
            bkL_sb = consts.tile([128, 2, 1], dt.float32, tag="bkl", name="bkl")
            bvA_sb = consts.tile([1, HG * DH], dt.bfloat16, tag="bva", name="bva")
            bvL_sb = consts.tile([1, N_LOC * DH], dt.bfloat16, tag="bvl", name="bvl")
            bo_sb = consts.tile([1, D], dt.bfloat16, tag="bo", name="bo")
            ones_sb = consts.tile([1, 128], dt.bfloat16, tag="ones", name="ones")
            y_acc = consts.tile([128, 4, D], dt.float32, tag="yacc", name="yacc")

            kat = [
                consts.tile([CP, T], dt.bfloat16, tag=f"kat{i}", name=f"kat{i}")
                for i in range(3)
            ]
            vtl = [
                consts.tile([128, KC, VG], dt.bfloat16, tag=f"vtl{i}", name=f"vtl{i}")
                for i in range(3)
            ]
            qat = [
                consts.tile([CP, TQ], dt.bfloat16, tag=f"qat{h}", name=f"qat{h}")
                for h in range(H)
            ]

            # ---- critical-path input DMAs first ----
            for (dst, src) in (
                (xk_sb, xkT),
                (xv_sb, xvT),
                (wkA_sb, wkA),
                (wvA_sb, wvA),
            ):
                nc.sync.dma_start(
                    out=dst, in_=src.rearrange("(a p) m -> p a m", p=128)
                )
            nc.sync.dma_start(out=bkA_sb, in_=bkAE.rearrange("(a p) o -> p a o", p=128))
            nc.sync.dma_start(out=bvA_sb, in_=bvAE)
            nc.vector.memset(ones_sb, 1.0)

            # ---- K^T projection: rows = head dims, cols = keys ----
            def kproj(w_sb, b_sb, dsts, dst_rows):
                for dc in range(len(dsts)):
                    for tc_i in range(4):
                        ps = psA.tile([128, 512], dt.float32, tag="mm", name="mmk")
                        for kc in range(8):
                            nc.tensor.matmul(
                                ps,
                                lhsT=w_sb[:, kc, ts(dc, 128)],
                                rhs=xk_sb[:, kc, ts(tc_i, 512)],
                                start=(kc == 0),
                                stop=(kc == 7),
                            )
                        stg = stp.tile(
                            [128, 512], dt.bfloat16, tag="kstage", name="kstage"
                        )
                        nc.vector.tensor_scalar_add(stg, ps, b_sb[:, dc, :])
                        nc.sync.dma_start(
                            out=dsts[dc][ds(dst_rows[dc], 128), ts(tc_i, 512)],
                            in_=stg,
                        )

            # ---- V projection: rows = keys, cols = head dims ----
            def vproj(w_sb, bv_row, nch, col_dsts):
                for tc_i in range(16):
                    ps = psA.tile([128, nch], dt.float32, tag="mm", name="mmv")
                    for kc in range(8):
                        nc.tensor.matmul(
                            ps,
                            lhsT=xv_sb[:, kc, ts(tc_i, 128)],
                            rhs=w_sb[:, kc, :],
                            start=(kc == 0),
                            stop=False,
                        )
                    nc.tensor.matmul(
                        ps, lhsT=ones_sb[:, :], rhs=bv_row, start=False, stop=True
                    )
                    stg = stp.tile([128, nch], dt.bfloat16, tag="vstage", name="vstage")
                    nc.vector.tensor_copy(stg, ps)
                    for (cl, cw, view, vrow) in col_dsts:
                        nc.sync.dma_start(
                            out=view[ds(vrow + 128 * tc_i, 128), :],
                            in_=stg[:, ds(cl, cw)],
                        )

            # ---- P1: sharded K/V projections -> two AllGather waves ----
            kproj(wkA_sb, bkA_sb, (kv_in[0], kv_in[1]), (0, 0))
            vin_view = [
                kv_in[w].rearrange("r (s n) -> (r s) n", n=128) for w in range(2)
            ]
            vproj(
                wvA_sb,
                bvA_sb,
                HG * DH,
                [(0, 128, vin_view[0], T), (128, 128, vin_view[1], T)],
            )
            for w in range(2):
                nc.gpsimd.collective_compute(
                    "AllGather",
                    Bypass,
                    ins=[kv_in[w].opt()],
                    outs=[kv_out[w].opt()],
                    replica_groups=RG,
                )

            # ---- remaining input DMAs (deprioritized behind the AG inputs) ----
            for (dst, src) in (
                (wq_sb, wqT),
                (xq_sb, xqT),
                (wkL_sb, wkL),
                (wvL_sb, wvL),
                (wo_sb, woT),
            ):
                nc.sync.dma_start(
                    out=dst, in_=src.rearrange("(a p) m -> p a m", p=128)
                )
            nc.sync.dma_start(out=bq_sb, in_=bqE.rearrange("(a p) o -> p a o", p=128))
            nc.sync.dma_start(out=bkL_sb, in_=bkLE.rearrange("(a p) o -> p a o", p=128))
            nc.sync.dma_start(out=bvL_sb, in_=bvLE)
            nc.sync.dma_start(out=bo_sb, in_=boE)
            for i in range(3):
                nc.sync.dma_start(out=kat[i][DH:CP, :], in_=kfE)
                nc.vector.memset(vtl[i][:, :, DH:VG], 1.0)

            # ---- P2: redundant local projections of heads 0..3 ----
            kproj(wkL_sb, bkL_sb, (k_loc, k_loc), (0, 128))
            vloc_view = v_loc.rearrange("r (s n) -> (r s) n", n=256)
            vproj(wvL_sb, bvL_sb, N_LOC * DH, [(0, 256, vloc_view, 0)])

            # ---- P3: Q projection (wq columns already in slot order) ----
            for dc in range(8):
                ps = psA.tile([128, TQ], dt.float32, tag="mm", name="mmq")
                for kc in range(8):
                    nc.tensor.matmul(
                        ps,
                        lhsT=wq_sb[:, kc, ts(dc, 128)],
                        rhs=xq_sb[:, kc, :],
                        start=(kc == 0),
                        stop=(kc == 7),
                    )
                stg = stp.tile([128, TQ], dt.bfloat16, tag="qstage", name="qstage")
                nc.vector.tensor_scalar_add(stg, ps, bq_sb[:, dc, :])
                nc.sync.dma_start(out=qat[2 * dc][0:DH, :], in_=stg[0:DH, :])
                nc.sync.dma_start(out=qat[2 * dc + 1][0:DH, :], in_=stg[DH:128, :])
            for h in range(H):
                nc.sync.dma_start(out=qat[h][DH:CP, :], in_=qfE[ds(AUG * h, AUG), :])

            vout_view = [
                kv_out[w].rearrange("r (s n) -> (r s) n", n=128) for w in range(2)
            ]

            # ---- P4: attention per slot ----
            # The out-projection of pair p is emitted in the middle of slot
            # 2p+2's supertile stream so the in-order PE queue never stalls
            # on the DVE/gpsimd normalization chain feeding cur_aot.
            def outproj(pair, aot_t):
                for qc in range(4):
                    for mc in range(2):
                        ps_y = psA.tile([128, 512], dt.float32, tag="mm", name="mmy")
                        nc.tensor.matmul(
                            ps_y,
                            lhsT=aot_t[:, ts(qc, 128)],
                            rhs=wo_sb[:, pair, ts(mc, 512)],
                            start=True,
                            stop=(pair != 0),
                        )
                        if pair == 0:
                            nc.tensor.matmul(
                                ps_y,
                                lhsT=ones_sb[:, :],
                                rhs=bo_sb[:, ds(512 * mc, 512)],
                                start=False,
                                stop=True,
                            )
                            nc.vector.tensor_copy(
                                y_acc[:, qc, ds(512 * mc, 512)], ps_y
                            )
                        else:
                            nc.vector.tensor_tensor(
                                out=y_acc[:, qc, ds(512 * mc, 512)],
                                in0=ps_y,
                                in1=y_acc[:, qc, ds(512 * mc, 512)],
                                op=Add,
                            )

            cur_aot = None
            pending = None
            for j in range(H):
                ka = kat[j % 3]
                vt = vtl[j % 3]
                if j < N_LOC:
                    ksrc = k_loc[ds(DH * j, DH), :]
                    vsrc = v_loc[:, ds(DH * j, DH)]
                else:
                    w = 0 if j < 10 else 1
                    r = 1 + (j - (4 if w == 0 else 10)) // 2
                    m = (j - (4 if w == 0 else 10)) % 2
                    ksrc = kv_out[w][ds(256 * r + DH * m, DH), :]
                    vsrc = vout_view[w][ds(4096 * r + T, T), ds(DH * m, DH)]
                nc.sync.dma_start(out=ka[0:DH, :], in_=ksrc)
                nc.sync.dma_start(
                    out=vt[:, :, 0:DH],
                    in_=vsrc.rearrange("(a p) m -> p a m", p=128),
                )
                ps_av = psAV.tile([VG, TQ], dt.float32, tag="av", name="av")
                for i in range(KC // 2):
                    ss = psS.tile([128, 2 * TQ], dt.float32, tag="s", name="s")
                    nc.tensor.matmul(
                        ss[:, 0:TQ],
                        lhsT=ka[:, ts(2 * i, 128)],
                        rhs=qat[j],
                        start=True,
                        stop=True,
                    )
                    nc.tensor.matmul(
                        ss[:, TQ : 2 * TQ],
                        lhsT=ka[:, ts(2 * i + 1, 128)],
                        rhs=qat[j],
                        start=True,
                        stop=True,
                    )
                    eh = ep.tile([128, 2 * TQ], dt.bfloat16, tag="ehat", name="ehat")
                    nc.scalar.activation(eh, ss, Exp)
                    nc.tensor.matmul(
                        ps_av,
                        lhsT=vt[:, 2 * i, :],
                        rhs=eh[:, 0:TQ],
                        start=(i == 0),
                        stop=False,
                    )
                    nc.tensor.matmul(
                        ps_av,
                        lhsT=vt[:, 2 * i + 1, :],
                        rhs=eh[:, TQ : 2 * TQ],
                        start=False,
                        stop=(i == KC // 2 - 1),
                    )
                    if i == 2 and pending is not None:
                        outproj(*pending)
                        pending = None
                # normalization: DVE reciprocal + gpsimd broadcast + DVE mul
                den = rrp.tile([1, TQ], dt.float32, tag="den", name="den")
                nc.vector.tensor_copy(den, ps_av[ds(DH, 1), :])
                rcp = rrp.tile([1, TQ], dt.float32, tag="rcp", name="rcp")
                nc.vector.reciprocal_approx_fast(rcp, den)
                rcpb = rrp.tile([1, TQ], dt.bfloat16, tag="rcpb", name="rcpb")
                nc.vector.tensor_copy(rcpb, rcp)
                rb = rbp.tile([DH, TQ], dt.bfloat16, tag="rb", name="rb")
                nc.gpsimd.partition_broadcast(rb, rcpb)
                if j % 2 == 0:
                    cur_aot = aop.tile([128, TQ], dt.bfloat16, tag="ao", name="ao")
                nc.vector.tensor_tensor(
                    out=cur_aot[ds(DH * (j % 2), DH), :],
                    in0=ps_av[0:DH, :],
                    in1=rb,
                    op=Mult,
                )
                if j % 2 == 1:
                    pending = (j // 2, cur_aot)
            outproj(*pending)

            # ---- P5: output DMA ----
            nc.sync.dma_start(
                out=outE.rearrange("(a p) m -> p a m", p=128), in_=y_acc
            )

    nc.compile()
    _state["nc"] = nc
    return nc


def _make_in_maps(inputs):
    _features()
    gs = float(np.float32(inputs["gamma"])) * DH ** -0.5
    delta = float(np.float32(inputs["delta"]))
    ap_ = np.asarray(inputs["alpha_pos"], np.float64)
    an_ = np.asarray(inputs["alpha_neg"], np.float64)

    # permute per-head blocks of Wq/bq/qfeat/Wo into slot order
    wqT_n = np.asarray(inputs["Wq"], np.float64).T * gs
    bq_n = np.asarray(inputs["bq"], np.float64) * gs
    wqTh = np.concatenate([wqT_n[:, DH * h : DH * (h + 1)] for h in PERM], 1).astype(BF)
    bqh = np.concatenate(
        [bq_n[DH * h : DH * (h + 1)] for h in PERM], 0
    ).astype(np.float32)[:, None]
    woT_n = np.ascontiguousarray(np.asarray(inputs["Wo"]).T)
    woTh = np.concatenate([woT_n[DH * h : DH * (h + 1), :] for h in PERM], 0).astype(BF)

    wkT_full = np.ascontiguousarray(np.asarray(inputs["Wk"]).T)
    bk_full = np.asarray(inputs["bk"], np.float32)
    wvT_full = np.ascontiguousarray(np.asarray(inputs["Wv"]).T)
    bv_full = np.asarray(inputs["bv"], np.float32)
    boh = np.asarray(inputs["bo"], np.float32)[None, :].astype(BF)

    wkLh = np.ascontiguousarray(wkT_full[:, : N_LOC * DH]).astype(BF)
    bkLh = np.ascontiguousarray(bk_full[: N_LOC * DH])[:, None]
    wvLh = np.ascontiguousarray(wvT_full[:, : N_LOC * DH]).astype(BF)
    bvLh = bv_full[None, : N_LOC * DH].astype(BF)

    phi, psi = _state["phi"], _state["psi"]
    frac = np.asarray(inputs["frac"], np.float64)

    in_maps = []
    for c in range(NCORES):
        b, p = c // 4, c % 4
        fb = frac[b]
        fq = fb[TQ * p : TQ * (p + 1)]
        kfeat = np.concatenate([_ev(phi, fb), _ev(psi, fb)], 0).astype(BF)
        qfeat = np.zeros((H * AUG, TQ), np.float64)
        for s, h in enumerate(PERM):
            a_h = delta * ap_[h] / NGRID
            b_h = -delta * an_[h] / NGRID
            qfeat[AUG * s : AUG * s + R] = a_h * _ev(psi, fq)
            qfeat[AUG * s + R : AUG * (s + 1)] = b_h * _ev(phi, fq)
        qfeat = qfeat.astype(BF)

        xq = np.asarray(inputs["query"])[b, TQ * p : TQ * (p + 1)]
        sl = slice(HG * DH * p, HG * DH * (p + 1))
        in_maps.append(
            {
                "xqT": np.ascontiguousarray(xq.T).astype(BF),
                "xkT": np.ascontiguousarray(np.asarray(inputs["key"])[b].T).astype(BF),
                "xvT": np.ascontiguousarray(
                    np.asarray(inputs["value"])[b].T
                ).astype(BF),
                "wqT": wqTh,
                "wkA": np.ascontiguousarray(wkT_full[:, sl]).astype(BF),
                "wkL": wkLh,
                "wvA": np.ascontiguousarray(wvT_full[:, sl]).astype(BF),
                "wvL": wvLh,
                "woT": woTh,
                "bq": bqh,
                "bkA": np.ascontiguousarray(bk_full[sl])[:, None],
                "bkL": bkLh,
                "bvA": bv_full[None, sl].astype(BF),
                "bvL": bvLh,
                "bo": boh,
                "kfeat": kfeat,
                "qfeat": qfeat,
            }
        )
    return in_maps


def _run(inputs, trace=False, **kw):
    from concourse.bass_utils import run_bass_kernel_spmd

    nc = _build()
    in_maps = _make_in_maps(inputs)
    res = run_bass_kernel_spmd(
        nc, in_maps, core_ids=list(range(NCORES)), trace=trace, **kw
    )
    out = np.zeros((B, T, D), np.float32)
    for c in range(NCORES):
        b, p = c // 4, c % 4
        out[b, TQ * p : TQ * (p + 1)] = res.results[c]["out"]
    return out, res


def kernel(**inputs):
    out, _ = _run(inputs)
    return out
